# revision 19
# baseline (speedup 1.0000x reference)
"""GCN layer (x @ W.T aggregated over sparse adjacency) on 8 TRN2 NeuronCores.

Math:  out[d,:] = sum_{e: row[e]==d} val[e] * (x[col[e],:] @ W.T) + bias
Equivalently (used here): out = (A @ x) @ W.T + bias   with A the sparse
edge_val-weighted adjacency.  Aggregating raw x first avoids materializing
the dense `support` table: x itself (cast to bf16) is the gather table.

Sharding: destination nodes are split across the 8 cores (6250 each); each
core receives the full x table (replicated, free at exec time) plus its own
edge metadata.  Edges are gathered at PAIR granularity (one 512B descriptor
fetches x rows 2j and 2j+1): pair indices stay below 2^15 so a single int16
index table covers all 50000 nodes with no lo/hi table split, and batches
need only ~3 gather calls each.  Per 128-edge chunk a single DVE
tensor_scalar builds a [128, 256] selector ("iota256" trick: din_adj =
dest-in-tile + 128*(col parity), so even-source edges match in columns
0..127 and odd-source edges in 128..255); two PE matmuls per chunk
accumulate both parities into the [feat, dest] PSUM aggregate.  Output rows
are written back with batched DMAs.

The dma_gather issue path is the measured bottleneck (~7 ns per descriptor,
independent of address locality and element size), so the kernel runs at
~edge-count x 7 ns with compute hidden underneath.
"""

import math

import numpy as np
import ml_dtypes

import concourse.bacc as bacc
import concourse.mybir as mybir
import concourse.tile as tile
from concourse.bass_utils import run_bass_kernel_spmd

# Problem constants (fixed by the harness).
N_NODES = 50000
N_EDGES = 800000
D = 128
C = 8                      # cores
NPC = N_NODES // C         # 6250 destination nodes per core
P = 128
T = math.ceil(NPC / P)     # 49 dest tiles per core
N_PAIRS = N_NODES // 2     # 25000 gather-table entries (int16-safe)
G_TILES = 4                # dest tiles per gather batch

BF16 = mybir.dt.bfloat16
F32 = mybir.dt.float32
I16 = mybir.dt.int16


def _ru(x, m):
    return (x + m - 1) // m * m


def _prep_host(edge_row, edge_col, edge_val, ramp=True):
    """Partition/sort/pad edges.  Returns the shared static structure plus
    per-core flat arrays (pair idx, adjusted dest-in-tile, val)."""
    er = np.asarray(edge_row).astype(np.int64)
    ec = np.asarray(edge_col).astype(np.int64)
    ev = np.asarray(edge_val).astype(np.float32)

    core = er // NPC
    dloc = er % NPC
    tl = dloc // P
    din = dloc % P

    # counts per (core, tile)
    cnt = np.zeros((C, T), np.int64)
    np.add.at(cnt, (core, tl), 1)

    # shared segment sizes (max over cores), rounded to 128
    n_pad = np.zeros(T, np.int64)
    for t in range(T):
        n_pad[t] = _ru(max(int(cnt[:, t].max()), 1), P)

    # Finer batches at the start shrink the pipeline ramp: the first
    # gathers are small, so compute starts sooner and hides under the
    # remaining ~95% of the gather stream.
    if ramp:
        batches = [[0], [1], [2, 3]]
        batches += [list(range(b, min(b + G_TILES, T)))
                    for b in range(4, T, G_TILES)]
    else:
        batches = [list(range(b, min(b + G_TILES, T)))
                   for b in range(0, T, G_TILES)]
    seg_off = np.zeros(T, np.int64)
    off = 0
    for t in range(T):
        seg_off[t] = off
        off += n_pad[t]
    L = off                               # total padded edges per core
    K = L // P                            # total 128-edge chunks

    # flat padded position of every edge: seg_off[tile] + rank in segment.
    # Edges within a segment are sorted by source id so gather descriptors
    # hit ascending HBM addresses.
    order = np.lexsort((ec, tl, core))
    so = seg_off[tl[order]]
    key = core[order] * T + tl[order]
    newgrp = np.ones(len(key), bool)
    newgrp[1:] = key[1:] != key[:-1]
    idxs = np.arange(len(key))
    grp_start = np.maximum.accumulate(np.where(newgrp, idxs, 0))
    rank = idxs - grp_start
    pos = so + rank

    idx_flat = np.zeros((C, L), np.int16)       # pad -> pair 0 (weight 0)
    dina_flat = np.full((C, L), -1.0, np.float32)
    val_flat = np.zeros((C, L), np.float32)
    oc = core[order]
    eco = ec[order]
    idx_flat[oc, pos] = (eco >> 1).astype(np.int16)
    # din_adj = din + 128*parity: selects the even/odd half of the selector
    dina_flat[oc, pos] = (din[order] + P * (eco & 1)).astype(np.float32)
    val_flat[oc, pos] = ev[order]

    return dict(
        n_pad=n_pad, batches=batches, seg_off=seg_off, L=L, K=K,
        idx_flat=idx_flat, dina_flat=dina_flat, val_flat=val_flat,
    )


def _wrap_idx(idx_flat_core):
    """[128, L//16] int16: idx j at partition j%16 (replicated to all 8
    groups of 16 partitions), free column j//16."""
    L = idx_flat_core.shape[0]
    out = np.zeros((P, L // 16), np.int16)
    out[:16, :] = idx_flat_core.reshape(L // 16, 16).T
    for g in range(1, 8):
        out[16 * g:16 * (g + 1), :] = out[:16, :]
    return out


def _build_program(st, repeat=1, skip_gather=False, skip_compute=False,
                   call=1536, mbufs=3, sp=False, stile_engine="vector",
                   sbufs=8, nqueues=4, abufs=4):
    """Emit the Bass/Tile program (shared by all 8 cores)."""
    n_pad, batches, seg_off = st["n_pad"], st["batches"], st["seg_off"]
    L, K = st["L"], st["K"]

    nc = bacc.Bacc("TRN2", target_bir_lowering=False,
                   num_swdge_queues=nqueues)
    xp_d = nc.dram_tensor("xp", [N_PAIRS, 2 * D], BF16, kind="ExternalInput")
    idx_d = nc.dram_tensor("idx", [P, L // 16], I16, kind="ExternalInput")
    din_d = nc.dram_tensor("din", [P, K], F32, kind="ExternalInput")
    val_d = nc.dram_tensor("val", [P, K], F32, kind="ExternalInput")
    wt_d = nc.dram_tensor("wt", [P, D], BF16, kind="ExternalInput")
    iota_d = nc.dram_tensor("iota", [P, 2 * P], BF16, kind="ExternalInput")
    bias_d = nc.dram_tensor("bias_row", [1, D], BF16, kind="ExternalInput")
    ones_d = nc.dram_tensor("ones_row", [1, P], BF16, kind="ExternalInput")
    out_d = nc.dram_tensor("out", [NPC, D], F32, kind="ExternalOutput")

    with tile.TileContext(nc) as tc:
        with (
            tc.tile_pool(name="const", bufs=1) as cpool,
            tc.tile_pool(name="msgs", bufs=mbufs) as mpool,
            tc.tile_pool(name="st", bufs=sbufs) as spool,
            tc.tile_pool(name="aggp", bufs=abufs, space="PSUM") as agg_pool,
            tc.tile_pool(name="outp", bufs=2, space="PSUM") as outp_pool,
            tc.tile_pool(name="aggs", bufs=3) as aggs_pool,
            tc.tile_pool(name="outs", bufs=3) as outs_pool,
        ):
            idx_sb = cpool.tile([P, L // 16], I16)
            din_sb = cpool.tile([P, K], F32)
            val_sb = cpool.tile([P, K], F32)
            wt_sb = cpool.tile([P, D], BF16)
            iota_sb = cpool.tile([P, 2 * P], BF16)
            bias_sb = cpool.tile([1, D], BF16)
            ones_sb = cpool.tile([1, P], BF16)
            nc.sync.dma_start(out=idx_sb[:], in_=idx_d[:])
            nc.sync.dma_start(out=din_sb[:], in_=din_d[:])
            nc.sync.dma_start(out=val_sb[:], in_=val_d[:])
            nc.sync.dma_start(out=wt_sb[:], in_=wt_d[:])
            nc.sync.dma_start(out=iota_sb[:], in_=iota_d[:])
            nc.sync.dma_start(out=bias_sb[:], in_=bias_d[:])
            nc.sync.dma_start(out=ones_sb[:], in_=ones_d[:])

            kb_max = max(int(sum(n_pad[t] for t in bt)) // P for bt in batches)

            def _emit_batch(bi, bt):
                boff = int(seg_off[bt[0]])
                blen = int(sum(n_pad[t] for t in bt))
                msgs = mpool.tile([P, kb_max, 2 * D], BF16, tag="msgs")
                if skip_gather:
                    nc.vector.memset(msgs[:], 0.0)
                else:
                    # pair-gathers cover the whole batch (incl. pad slack:
                    # idx 0 there -> pair 0, killed by val=0).
                    # single_packet=True corrupts/hangs the device; calls of
                    # 4096+ descriptors stall the SWDGE ring (~+50us/iter).
                    CALL = call
                    for so in range(0, blen, CALL):
                        sl = min(CALL, blen - so)
                        nc.gpsimd.dma_gather(
                            out_ap=msgs[:, so // P:(so + sl) // P, :],
                            in_ap=xp_d[:],
                            idxs_ap=idx_sb[:, (boff + so) // 16:
                                           (boff + so + sl) // 16],
                            num_idxs=sl,
                            num_idxs_reg=sl,
                            elem_size=2 * D,
                            single_packet=sp,
                            queue_num=qrr[0] % nqueues,
                        )
                        qrr[0] += 1
                if skip_compute:
                    return
                outs = outs_pool.tile([P, len(bt), D], F32, tag="outs")
                for ti, t in enumerate(bt):
                    kt = int(n_pad[t]) // P
                    g0 = int(seg_off[t]) // P       # global chunk idx
                    c0 = (int(seg_off[t]) - boff) // P  # within batch
                    aggp = agg_pool.tile([P, P], F32, tag="aggp")
                    for q in range(kt):
                        stile = spool.tile([P, 2 * P], BF16, tag="st")
                        eng = getattr(nc, stile_engine)
                        eng.tensor_scalar(
                            out=stile[:],
                            in0=iota_sb[:],
                            scalar1=din_sb[:, g0 + q:g0 + q + 1],
                            scalar2=val_sb[:, g0 + q:g0 + q + 1],
                            op0=mybir.AluOpType.is_equal,
                            op1=mybir.AluOpType.mult,
                        )
                        nc.tensor.matmul(
                            out=aggp[:],
                            lhsT=msgs[:, c0 + q, 0:D],
                            rhs=stile[:, 0:P],
                            start=(q == 0),
                            stop=False,
                        )
                        nc.tensor.matmul(
                            out=aggp[:],
                            lhsT=msgs[:, c0 + q, D:2 * D],
                            rhs=stile[:, P:2 * P],
                            start=False,
                            stop=(q == kt - 1),
                        )
                    # aggp = agg^T [feat x dest]; cast to bf16 and transform
                    aggs = aggs_pool.tile([P, P], BF16, tag="aggs")
                    nc.scalar.copy(out=aggs[:], in_=aggp[:])
                    outp = outp_pool.tile([P, D], F32, tag="outp")
                    nc.tensor.matmul(out=outp[:], lhsT=aggs[:], rhs=wt_sb[:],
                                     start=True, stop=False)
                    nc.tensor.matmul(out=outp[:], lhsT=ones_sb[:],
                                     rhs=bias_sb[:], start=False, stop=True)
                    nc.scalar.copy(out=outs[:, ti, :], in_=outp[:])
                # batched output write: row r of the batch = outs[r%128, r//128]
                r0 = bt[0] * P
                rows = min(NPC, (bt[-1] + 1) * P) - r0
                if rows == len(bt) * P:
                    hbm = out_d[r0:r0 + rows, :].rearrange(
                        "(c p) f -> p c f", p=P)
                    nc.sync.dma_start(out=hbm, in_=outs[:])
                else:
                    nfull = rows // P
                    if nfull:
                        hbm = out_d[r0:r0 + nfull * P, :].rearrange(
                            "(c p) f -> p c f", p=P)
                        nc.sync.dma_start(out=hbm, in_=outs[:, :nfull, :])
                    rem = rows - nfull * P
                    if rem:
                        nc.sync.dma_start(
                            out=out_d[r0 + nfull * P:r0 + rows, :],
                            in_=outs[:rem, nfull, :])

            qrr = [0]

            def body():
                for bi, bt in enumerate(batches):
                    _emit_batch(bi, bt)

            if repeat > 1:
                with tc.For_i(0, repeat, 1):
                    body()
            else:
                body()
    nc.compile()
    return nc


def make_in_maps(x, W, bias, st):
    x32 = np.asarray(x, np.float32)
    xp = x32.astype(ml_dtypes.bfloat16).reshape(N_PAIRS, 2 * D)
    wt = np.ascontiguousarray(np.asarray(W, np.float32).T).astype(
        ml_dtypes.bfloat16)                                   # [i, o]
    iota = np.tile(np.arange(2 * P, dtype=np.float32), (P, 1)).astype(
        ml_dtypes.bfloat16)
    bias_row = np.asarray(bias, np.float32)[None, :].astype(ml_dtypes.bfloat16)
    ones_row = np.ones((1, P), ml_dtypes.bfloat16)

    din_cols = st["dina_flat"].reshape(C, st["K"], P).transpose(0, 2, 1)
    val_cols = st["val_flat"].reshape(C, st["K"], P).transpose(0, 2, 1)

    in_maps = []
    for c in range(C):
        in_maps.append({
            "xp": xp,
            "idx": _wrap_idx(st["idx_flat"][c]),
            "din": np.ascontiguousarray(din_cols[c]),
            "val": np.ascontiguousarray(val_cols[c]),
            "wt": wt, "iota": iota, "bias_row": bias_row,
            "ones_row": ones_row,
        })
    return in_maps


def kernel(x, edge_row, edge_col, edge_val, W, bias):
    st = _prep_host(edge_row, edge_col, edge_val)
    nc = _build_program(st)
    in_maps = make_in_maps(x, W, bias, st)
    res = run_bass_kernel_spmd(nc, in_maps, core_ids=list(range(C)))
    out = np.concatenate([res.results[c]["out"] for c in range(C)], axis=0)
    return out.astype(np.float32)


if __name__ == "__main__":
    rng = np.random.default_rng(0)
    x = rng.standard_normal((N_NODES, D), dtype=np.float32)
    er = rng.integers(0, N_NODES, N_EDGES)
    ec = rng.integers(0, N_NODES, N_EDGES)
    ev = rng.random(N_EDGES, dtype=np.float32)
    W = rng.standard_normal((D, D), dtype=np.float32) / np.sqrt(D)
    b = np.zeros(D, np.float32)
    out = kernel(x, er, ec, ev, W, b)
    print(out.shape, out.dtype)
